# revision 48
# baseline (speedup 1.0000x reference)
"""HOSVD aggregator kernel for Trainium2 (8 NeuronCores, data-parallel over N).

Reference computation (per node n):
    p_d[n, :]  = ns[n, d, :] @ U_w[d].T + U_b[d]          d in {0, 1}
    r[n, c]    = sum_{a,b} G[a,b,c] * p_0[n,a] * p_1[n,b]
    out[n, :]  = r[n, :] @ U_out.T

Per 512-node group (4 tiles of 128 nodes), all in bf16 except PSUM accum:
  - one SWDGE cast-load (f32->bf16) into [p, (d k t q)] layout
  - ONE DmaTransposeAnt -> nst[i', ((d k t), n')]  (x with i on partitions)
  - projection: col-tiled PE matmuls (tile_position=(0,32t)) accumulate
    p01_ps[(t r), (d n')]; bias enters as a K=1 ones-row matmul
  - tucker: row-strip matmuls (tile_position=(32t,0)): stationary = p0T strip
    (already at partitions 32t..32t+31), moving = replicated G -> t[n', (c b)]
  - DVE: w = t (PSUM f32) * p1 broadcast (bf16)  -> w_sb bf16
  - GPSIMD: r[n', c] = sum_b w  (keeps DVE free for the muls)
  - PE transpose r -> rT strips, then row-strip output matmuls vs U_out
"""

import os
import sys

import numpy as np

for _p in ("/opt/trn_rl_repo", "/root/.axon_site/_ro/trn_rl_repo"):
    if os.path.isdir(_p) and _p not in sys.path:
        sys.path.insert(0, _p)

N = 200000
DEG = 2
IN_SIZE = 256
RANK = 32
OUT_SIZE = 256
NCORES = 8
TILE = 128
GT = 4                       # tiles per group
GROUP_NODES = TILE * GT      # 512

_CACHE = {}


def _build_bass(npad: int, bench_loops: int = 1):
    import concourse.bass as bass
    import concourse.tile as tile
    from concourse import bacc, mybir

    groups = npad // GROUP_NODES
    assert groups * GROUP_NODES == npad

    # Bacc (not plain Bass): its finalize() runs generate_event_semaphores,
    # which legalizes multi-semaphore joins down to TRN2's 1-wait-per-
    # instruction limit.
    nc = bacc.Bacc("TRN2", debug=False)

    f32 = mybir.dt.float32
    bf16 = mybir.dt.bfloat16

    ns_d = nc.dram_tensor("ns", [npad, DEG, IN_SIZE], f32, kind="ExternalInput")
    # packed bf16 consts:
    # [128, 128 uwt | 1024 gmat4 | 256 uout4 | 128 ident | 64 bias(row 0)]
    wb_d = nc.dram_tensor("wconst_b", [128, 1600], bf16, kind="ExternalInput")
    # bf16 store halves HBM write traffic; host casts back to f32
    out_d = nc.dram_tensor("out", [npad, OUT_SIZE], bf16, kind="ExternalOutput")

    from contextlib import ExitStack

    with tile.TileContext(nc) as tc, ExitStack() as ctx, nc.allow_low_precision(
        reason="bf16 pipeline validated against fp32 reference (rel tol 2e-2)"
    ):
        const = ctx.enter_context(tc.tile_pool(name="const", bufs=1))
        cb = const.tile([128, 1600], bf16)
        # Single const DMA: the first PE instruction then needs only ONE
        # sync wait (TRN2: at most one wait per instruction).
        nc.sync.dma_start(cb[:], wb_d.ap())
        uwt = cb[:, 0:128]                     # [i', (d k r)]
        gmat4 = cb[:, 128:1152]                # [(t a), (c b)] replicated
        uout4 = cb[:, 1152:1408]               # [(t c), s] replicated
        ident = cb[:, 1408:1536]
        biasw = cb[0:1, 1536:1600]             # [1, (d r)] = U_b
        ones_sb = const.tile([1, TILE], bf16)
        nc.gpsimd.memset(ones_sb[:], 1.0)

        ns_pool = ctx.enter_context(tc.tile_pool(name="ns", bufs=3))
        nst_pool = ctx.enter_context(tc.tile_pool(name="nst", bufs=2))
        p_pool = ctx.enter_context(tc.tile_pool(name="p", bufs=2))
        w_pool = ctx.enter_context(tc.tile_pool(name="w", bufs=4))
        r_pool = ctx.enter_context(tc.tile_pool(name="r", bufs=3))
        o_pool = ctx.enter_context(tc.tile_pool(name="o", bufs=2))

        ps_p = ctx.enter_context(tc.tile_pool(name="ps_p", bufs=2, space="PSUM"))
        ps_t = ctx.enter_context(tc.tile_pool(name="ps_t", bufs=3, space="PSUM"))
        ps_sm = ctx.enter_context(tc.tile_pool(name="ps_sm", bufs=1, space="PSUM"))
        ps_o = ctx.enter_context(tc.tile_pool(name="ps_o", bufs=2, space="PSUM"))

        Copy = mybir.ActivationFunctionType.Copy

        # Prologue: absorb the const-DMA semaphore into the PE clock early.
        dummy_ps = ps_sm.tile([128, 128], bf16, tag="sm")
        nc.tensor.transpose(dummy_ps[:], ident, ident)

        uwtr = uwt.rearrange("p (d k r) -> p d k r", d=DEG, k=2)

        loop_ctx = (
            tc.For_i(0, bench_loops) if bench_loops > 1 else None
        )
        if loop_ctx is not None:
            ctx.enter_context(loop_ctx)

        ns_tiles = {}
        nst_tiles = {}

        def issue_load(g):
            # cast f32 -> bf16; t-outer keeps each (t, p) DRAM row one
            # contiguous SBUF run (fewest SWDGE descriptors).
            n0 = g * GROUP_NODES
            ns_sb = ns_pool.tile([128, DEG * 2 * GT * 128], bf16, tag="ns")
            nc.gpsimd.dma_start(
                ns_sb[:].rearrange("p (t d k q) -> p t d k q", t=GT, d=DEG, k=2),
                ns_d.ap()[n0 : n0 + GROUP_NODES].rearrange(
                    "(t p) d (k q) -> p t d k q", p=128, k=2
                ),
            )
            ns_tiles[g] = ns_sb

        def issue_transpose(g):
            # ONE xbar transpose: nst[i', ((t d k), n')]
            nst = nst_pool.tile([128, DEG * 2 * GT * 128], bf16, tag="nst")
            nc.sync.dma_start(
                nst[:].rearrange("p (m q) -> p m q", m=DEG * 2 * GT),
                ns_tiles.pop(g)[:].rearrange("p (m q) -> p m q", m=DEG * 2 * GT),
                transpose=True,
            )
            nst_tiles[g] = nst

        def back_end(r_g, n0):
            # rT strips, then output matmuls; emitted one group late so the
            # PE queue never blocks the next group's front end on this
            # group's elementwise pipeline.
            rt_ps = ps_sm.tile([128, 128], bf16, tag="sm")
            nc.tensor.transpose(rt_ps[:], r_g[:], ident)
            rt_sb = r_pool.tile([128, 128], bf16, tag="rt")
            nc.scalar.activation(rt_sb[:], rt_ps[:], Copy)

            o_sb = o_pool.tile([128, GT * OUT_SIZE], bf16, tag="o")
            for t in range(GT):
                o_ps = ps_o.tile([128, OUT_SIZE], f32, tag="o")
                nc.tensor.matmul(
                    o_ps[:],
                    rt_sb[32 * t : 32 * t + 32, :],
                    uout4[32 * t : 32 * t + 32, :],
                    start=True,
                    stop=True,
                    tile_position=(32 * t, 0),
                )
                nc.scalar.activation(
                    o_sb[:, t * OUT_SIZE : (t + 1) * OUT_SIZE], o_ps[:], Copy
                )

            nc.sync.dma_start(
                out_d.ap()[n0 : n0 + GROUP_NODES].rearrange(
                    "(t p) s -> p t s", p=128
                ),
                o_sb[:].rearrange("p (t s) -> p t s", t=GT),
            )

        for g in range(min(2, groups)):
            issue_load(g)
        issue_transpose(0)
        pending = None

        for g in range(groups):
            n0 = g * GROUP_NODES
            # software pipeline: next loads/transposes go FIRST so their
            # queue slots aren't stuck behind this group's late stages.
            if g + 2 < groups:
                issue_load(g + 2)
            if g + 1 < groups:
                issue_transpose(g + 1)

            nstr = nst_tiles.pop(g)[:].rearrange(
                "p (t d k q) -> p t d k q", t=GT, d=DEG, k=2
            )

            # ---- projection psum: [.., 0:128] p0T col-tiled strips,
            #      [.., 128:256] p1 directly in N-layout [n', (t b)] ----
            p01_ps = ps_p.tile([128, 2 * TILE], f32, tag="p01")
            for t in range(GT):
                reg = p01_ps[32 * t : 32 * t + 32, 0:TILE]
                for k in range(2):
                    nc.tensor.matmul(
                        reg,
                        uwtr[:, 0, k, :],
                        nstr[:, t, 0, k, :],
                        start=(k == 0),
                        stop=False,
                        tile_position=(0, 32 * t),
                    )
                # bias as rank-1: += U_b[0, r] * 1
                nc.tensor.matmul(
                    reg,
                    biasw[:, 0:RANK],
                    ones_sb[:],
                    start=False,
                    stop=True,
                    tile_position=(0, 32 * t),
                )
            for t in range(GT):
                # p1 N-layout: stationary = x1T tile, moving = U_w[1]
                reg = p01_ps[:, TILE + 32 * t : TILE + 32 * t + 32]
                for k in range(2):
                    nc.tensor.matmul(
                        reg,
                        nstr[:, t, 1, k, :],
                        uwtr[:, 1, k, :],
                        start=(k == 0),
                        stop=False,
                    )
                nc.tensor.matmul(
                    reg,
                    ones_sb[:],
                    biasw[:, RANK : 2 * RANK],
                    start=False,
                    stop=True,
                )

            # ---- to SBUF bf16; two copies so the tucker (p0 strips) does
            # not wait for the p1 matmuls' accumulation to finish ----
            p01_sb = p_pool.tile([128, 2 * TILE], bf16, tag="p01sb")
            nc.scalar.activation(p01_sb[:, 0:TILE], p01_ps[:, 0:TILE], Copy)
            nc.scalar.activation(
                p01_sb[:, TILE : 2 * TILE], p01_ps[:, TILE : 2 * TILE], Copy
            )

            # ---- per tile: tucker matmul + DVE mul + b-reduce ----
            # w layout is (b, c): contiguous halves fold b on GPSIMD via
            # tensor_tensor adds; DVE reduces via a transposed view.
            r_g = r_pool.tile([128, 128], bf16, tag="rg")
            for t in range(GT):
                w_sb = w_pool.tile([128, RANK * RANK], bf16, tag="w")
                for h in range(2):  # N<=512 fp32 per PSUM bank
                    t_ph = ps_t.tile([128, 512], f32, tag="t")
                    nc.tensor.matmul(
                        t_ph[:],
                        p01_sb[32 * t : 32 * t + 32, 0:TILE],
                        gmat4[32 * t : 32 * t + 32, h * 512 : (h + 1) * 512],
                        start=True,
                        stop=True,
                        tile_position=(32 * t, 0),
                    )
                    # half h covers b in [16h, 16h+16)
                    p1b = (
                        p01_sb[
                            :, TILE + 32 * t + 16 * h : TILE + 32 * t + 16 * h + 16
                        ]
                        .unsqueeze(2)
                        .broadcast_to([128, 16, RANK])
                    )
                    nc.vector.tensor_mul(
                        w_sb[:, h * 512 : (h + 1) * 512].rearrange(
                            "p (b c) -> p b c", b=16
                        ),
                        t_ph[:].rearrange("p (b c) -> p b c", b=16),
                        p1b,
                    )
                rt_out = r_g[:, 32 * t : 32 * t + 32]
                # fold b by contiguous halves 32->16 on DVE (2x bf16 mode),
                # then 16->1 on GPSIMD (keeps DVE free for the next mul)
                fold = w_pool.tile([128, 512], bf16, tag="fold")
                nc.vector.tensor_add(fold[:], w_sb[:, 0:512], w_sb[:, 512:1024])
                for sz in (256, 128, 64):
                    nc.gpsimd.tensor_add(
                        fold[:, 0:sz], fold[:, 0:sz], fold[:, sz : 2 * sz]
                    )
                nc.gpsimd.tensor_add(rt_out, fold[:, 0:32], fold[:, 32:64])

            if pending is not None:
                back_end(*pending)
            pending = (r_g, n0)

        back_end(*pending)

    nc.finalize()
    return nc


def _prep_weights(G, U_w, U_b, U_out):
    import ml_dtypes

    # uwt[i', (d, k, r)] = U_w[d, r, k*128 + i']
    wuwt = U_w.reshape(DEG, RANK, 2, 128).transpose(3, 0, 2, 1).reshape(128, 128)
    # gmat[a, (b, c)] = G[a, b, c], replicated on 4 partition strips
    wgmat = G.reshape(RANK, RANK * RANK)
    wgmat4 = np.tile(wgmat, (4, 1))
    # uout4[(t c), s] = U_out[s, c]
    wuout4 = np.tile(U_out.T, (4, 1))
    wb = np.zeros((128, 1600), dtype=np.float32)
    wb[:, 0:128] = wuwt
    wb[:, 128:1152] = wgmat4
    wb[:, 1152:1408] = wuout4
    wb[:, 1408:1536] = np.eye(128, dtype=np.float32)
    wb[0, 1536:1600] = U_b.reshape(DEG * RANK)
    return np.ascontiguousarray(wb.astype(ml_dtypes.bfloat16))


def kernel(neighbour_states, G, U_w, U_b, U_out):
    """Data-parallel over nodes across 8 NeuronCores (Bass/Tile kernel)."""
    return kernel_bass(neighbour_states, G, U_w, U_b, U_out)


def kernel_bass(neighbour_states, G, U_w, U_b, U_out, trace=False):
    from concourse.bass_utils import run_bass_kernel_spmd

    ns = np.asarray(neighbour_states, dtype=np.float32)
    n_total = ns.shape[0]
    npc = (n_total + NCORES - 1) // NCORES
    npad = ((npc + GROUP_NODES - 1) // GROUP_NODES) * GROUP_NODES

    key = ("nc", npad)
    if key not in _CACHE:
        _CACHE[key] = _build_bass(npad)
    nc = _CACHE[key]

    wb = _prep_weights(
        np.asarray(G, np.float32),
        np.asarray(U_w, np.float32),
        np.asarray(U_b, np.float32),
        np.asarray(U_out, np.float32),
    )

    in_maps = []
    for c in range(NCORES):
        lo = c * npc
        hi = min(lo + npc, n_total)
        shard = np.zeros((npad, DEG, IN_SIZE), dtype=np.float32)
        shard[: hi - lo] = ns[lo:hi]
        in_maps.append({"ns": shard, "wconst_b": wb})

    res = run_bass_kernel_spmd(nc, in_maps, core_ids=list(range(NCORES)))
    outs = []
    for c in range(NCORES):
        lo = c * npc
        hi = min(lo + npc, n_total)
        outs.append(res.results[c]["out"][: hi - lo].astype(np.float32))
    return np.concatenate(outs, axis=0)
